# revision 12
# baseline (speedup 1.0000x reference)
"""Trainium2 Bass kernel for tanh-attention (nn_Attention_50362786513376).

reference:
  q = (x @ Wq.T) * dk^-0.5 ; k = x @ Wk.T ; v = x        (heads = 8, dk = 64)
  out = tanh(q k^T) v   per (batch, head),  merged back to [b, n, dim]

Sharding: 8 cores = 4 batches x 2 head-halves (4 heads per core).
Host pre-work (free, exact): transpose x[b] -> xT, slice v channels, slice +
scale + transpose weights. Device per core:
  Q^T = WqT.T @ xT, K^T = WkT.T @ xT     (ct-outer over 8 PSUM banks so the
                                          matmuls chase the chunked xT DMA)
  per head pair p, i-half, j-tile:  S^T[j,i] = K^T.T Q^T  (row-packed pairs)
  tanh on ScalarE PSUM->SBUF (the throughput bottleneck: n^2*h*b/8 elements)
  out^T[d,i] += v[j,:].T @ tanh(S^T)     (accumulated in PSUM over j)
Host post-work: out[b,:,half] = outT.T
"""
import numpy as np

HEADS = 8
DK = 64
B = 4
N = 2048
DIM = 512
SCALE = DK ** (-0.5)
NCORES = 8
HALF = DIM // 2  # 256 channels per core (4 heads)

_built = None
_built_cfg = None
PROJ_DTYPE = "f16"   # "f32r" | "f16"  (x / weights / projection matmuls)
ATTN_DTYPE = "f16"    # "f32r" | "f16"  (Q^T/K^T, qk mms)
V_DTYPE = "f16"       # "f16" | "bf16"  (tanh output + v operand of the AV mms)
TRACE = False
TRACE_KW = {}


def _build():
    from contextlib import ExitStack

    import concourse.tile as tile
    from concourse import bacc, mybir

    F32 = mybir.dt.float32
    DT = {"f32r": mybir.dt.float32r, "f16": mybir.dt.float16,
          "bf16": mybir.dt.bfloat16}
    PROJ_DT = DT[PROJ_DTYPE]
    ATTN_DT = DT[ATTN_DTYPE]
    V_DT = DT[V_DTYPE]
    NPDT = {"f32r": np.float32, "f16": np.float16}
    Tanh = mybir.ActivationFunctionType.Tanh

    nc = bacc.Bacc("TRN2", target_bir_lowering=False, debug=False,
                   num_devices=NCORES)
    xT_ap = nc.dram_tensor("xT", [DIM, N], PROJ_DT, kind="ExternalInput").ap()
    xv_ap = nc.dram_tensor("xv", [N, HALF], V_DT, kind="ExternalInput").ap()
    wqT_ap = nc.dram_tensor("wqT", [DIM, HALF], PROJ_DT,
                            kind="ExternalInput").ap()
    wkT_ap = nc.dram_tensor("wkT", [DIM, HALF], PROJ_DT,
                            kind="ExternalInput").ap()
    outT_ap = nc.dram_tensor("outT", [HALF, N], F32, kind="ExternalOutput").ap()

    NT = N // 512          # 4 t-chunks of 512
    NJ = N // 128          # 16 j-tiles

    with tile.TileContext(nc) as tc:
        with ExitStack() as ctx:
            const = ctx.enter_context(tc.tile_pool(name="const", bufs=1))
            qk_pool = ctx.enter_context(tc.tile_pool(name="qk", bufs=1))
            tanh_pool = ctx.enter_context(tc.tile_pool(name="tanh", bufs=6))
            stg_pool = ctx.enter_context(tc.tile_pool(name="stg", bufs=4))

            # ---- load inputs (xT first: projections chase its chunks) ----
            xT_sb = const.tile([128, 4 * N], PROJ_DT)
            for ct in range(4):
                nc.sync.dma_start(xT_sb[:, ct * N:(ct + 1) * N],
                                  xT_ap[ct * 128:(ct + 1) * 128, :])
            # wqT/wkT [512, 256] -> [128, 4*256] (c-tile ct at cols ct*256)
            wq_sb = const.tile([128, 4 * HALF], PROJ_DT)
            wk_sb = const.tile([128, 4 * HALF], PROJ_DT)
            for w_sb, w_ap in ((wq_sb, wqT_ap), (wk_sb, wkT_ap)):
                for ct in range(4):
                    nc.sync.dma_start(w_sb[:, ct * HALF:(ct + 1) * HALF],
                                      w_ap[ct * 128:(ct + 1) * 128, :])
            # xv [2048, 256] -> 16 contiguous j-tiles [128, 256]
            xv_sb = []
            for j in range(16):
                t = const.tile([128, HALF], V_DT, tag=f"xv{j}", name=f"xv{j}")
                nc.sync.dma_start(t[:], xv_ap[j * 128:(j + 1) * 128, :])
                xv_sb.append(t)

            # ---- projections: QT/KT pair tiles [128, 2048] ----
            # ct-outer over 8 psum banks (2 pairs x 4 t-chunks per projection)
            QT = [qk_pool.tile([128, N], ATTN_DT, tag=f"qt{p}", name=f"qt{p}")
                  for p in range(2)]
            KT = [qk_pool.tile([128, N], ATTN_DT, tag=f"kt{p}", name=f"kt{p}")
                  for p in range(2)]
            with tc.tile_pool(name="ps_proj", bufs=8, space="PSUM") as ps_proj:
                for dst, w_sb in ((QT, wq_sb), (KT, wk_sb)):
                    ps = {(p, t4): ps_proj.tile([128, 512], F32, tag="proj",
                                                name="proj_ps")
                          for p in range(2) for t4 in range(NT)}
                    for ct in range(4):
                        for p in range(2):
                            lhsT = w_sb[:, ct * HALF + p * 128:
                                        ct * HALF + (p + 1) * 128]
                            for t4 in range(NT):
                                rhs = xT_sb[:, ct * N + t4 * 512:
                                            ct * N + t4 * 512 + 512]
                                nc.tensor.matmul(ps[(p, t4)][:], lhsT, rhs,
                                                 start=(ct == 0),
                                                 stop=(ct == 3))
                    for p in range(2):
                        for t4 in range(NT):
                            nc.vector.tensor_copy(
                                dst[p][:, t4 * 512:(t4 + 1) * 512],
                                ps[(p, t4)][:])

            # ---- attention ----
            ps_S = ctx.enter_context(
                tc.tile_pool(name="ps_S", bufs=2, space="PSUM"))
            ps_acc = ctx.enter_context(
                tc.tile_pool(name="ps_acc", bufs=4, space="PSUM"))

            stg = {}
            for p in range(2):
                for par in range(2):
                    stg[(p, par)] = stg_pool.tile([64, N], F32, tag="stg",
                                                  name="stg")
            for p in range(2):
                for ih in range(2):          # i-half: i cols ih*1024..+1024
                    acc = [[ps_acc.tile([64, 512], F32, tag="acc", name="acc")
                            for ic in range(2)] for par in range(2)]
                    for j in range(NJ):
                        for ic in range(2):  # i-chunk within half
                            i0 = ih * 1024 + ic * 512
                            S = ps_S.tile([128, 1024], F32, tag="S", name="S")
                            # row-packed pair: head parity 0 on PE rows 0-63,
                            # parity 1 on rows 64-127
                            nc.tensor.matmul(
                                S[:, 0:512],
                                KT[p][0:64, j * 128:(j + 1) * 128],
                                QT[p][0:64, i0:i0 + 512],
                                start=True, stop=True, tile_position=(0, 0))
                            nc.tensor.matmul(
                                S[:, 512:1024],
                                KT[p][64:128, j * 128:(j + 1) * 128],
                                QT[p][64:128, i0:i0 + 512],
                                start=True, stop=True, tile_position=(64, 0))
                            T = tanh_pool.tile([128, 1024], V_DT, tag="T",
                                               name="T")
                            nc.scalar.activation(T[:], S[:], Tanh)
                            for par in range(2):
                                lh = 2 * p + par
                                v = xv_sb[j][:, lh * 64:(lh + 1) * 64]
                                nc.tensor.matmul(
                                    acc[par][ic][:],
                                    v,
                                    T[:, par * 512:(par + 1) * 512],
                                    start=(j == 0), stop=(j == NJ - 1))
                    for par in range(2):
                        for ic in range(2):
                            sl = (ih * 2 + ic) * 512
                            nc.vector.tensor_copy(stg[(p, par)][:, sl:sl + 512],
                                                  acc[par][ic][:])
                for par in range(2):
                    lh = 2 * p + par
                    nc.sync.dma_start(outT_ap[lh * 64:(lh + 1) * 64, :],
                                      stg[(p, par)][:])

    nc.compile()
    return nc


def _get_built():
    global _built, _built_cfg
    cfg = (PROJ_DTYPE, ATTN_DTYPE, V_DTYPE)
    if _built is None or _built_cfg != cfg:
        _built = _build()
        _built_cfg = cfg
    return _built


def kernel(x, Wq, Wk):
    from concourse.bass_utils import run_bass_kernel_spmd

    x = np.asarray(x, dtype=np.float32)
    Wq = np.asarray(Wq, dtype=np.float32)
    Wk = np.asarray(Wk, dtype=np.float32)

    import ml_dtypes
    proj_np = np.float16 if PROJ_DTYPE == "f16" else np.float32
    v_np = {"f16": np.float16, "bf16": ml_dtypes.bfloat16}[V_DTYPE]

    nc = _get_built()
    in_maps = []
    for c in range(NCORES):
        b, half = c // 2, c % 2
        sl = slice(half * HALF, (half + 1) * HALF)
        in_maps.append({
            "xT": np.ascontiguousarray(x[b].T).astype(proj_np),
            "xv": np.ascontiguousarray(x[b][:, sl]).astype(v_np),
            "wqT": np.ascontiguousarray((SCALE * Wq[sl, :]).T).astype(proj_np),
            "wkT": np.ascontiguousarray(Wk[sl, :].T).astype(proj_np),
        })
    res = run_bass_kernel_spmd(nc, in_maps, core_ids=list(range(NCORES)),
                               trace=TRACE, **TRACE_KW)
    out = np.empty((B, N, DIM), np.float32)
    for c in range(NCORES):
        b, half = c // 2, c % 2
        out[b, :, half * HALF:(half + 1) * HALF] = res.results[c]["outT"].T
    if TRACE:
        kernel.last_results = res
    return out


# revision 13
# speedup vs baseline: 1.0957x; 1.0957x over previous
"""Trainium2 Bass kernel for tanh-attention (nn_Attention_50362786513376).

reference:
  q = (x @ Wq.T) * dk^-0.5 ; k = x @ Wk.T ; v = x        (heads = 8, dk = 64)
  out = tanh(q k^T) v   per (batch, head),  merged back to [b, n, dim]

Sharding: 8 cores = 4 batches x 2 head-halves (4 heads per core).
Host pre-work (free, exact): transpose x[b] -> xT, slice v channels, slice +
scale + transpose weights. Device per core:
  Q^T = WqT.T @ xT, K^T = WkT.T @ xT     (ct-outer over 8 PSUM banks so the
                                          matmuls chase the chunked xT DMA)
  per head pair p, i-half, j-tile:  S^T[j,i] = K^T.T Q^T  (row-packed pairs)
  tanh on ScalarE PSUM->SBUF (the throughput bottleneck: n^2*h*b/8 elements)
  out^T[d,i] += v[j,:].T @ tanh(S^T)     (accumulated in PSUM over j)
Host post-work: out[b,:,half] = outT.T
"""
import numpy as np

HEADS = 8
DK = 64
B = 4
N = 2048
DIM = 512
SCALE = DK ** (-0.5)
NCORES = 8
HALF = DIM // 2  # 256 channels per core (4 heads)

_built = None
_built_cfg = None
PROJ_DTYPE = "f16"   # "f32r" | "f16"  (x / weights / projection matmuls)
ATTN_DTYPE = "f16"    # "f32r" | "f16"  (Q^T/K^T, qk mms)
V_DTYPE = "f16"       # "f16" | "bf16"  (tanh output + v operand of the AV mms)
TRACE = False
TRACE_KW = {}


def _build():
    from contextlib import ExitStack

    import concourse.tile as tile
    from concourse import bacc, mybir

    F32 = mybir.dt.float32
    DT = {"f32r": mybir.dt.float32r, "f16": mybir.dt.float16,
          "bf16": mybir.dt.bfloat16}
    PROJ_DT = DT[PROJ_DTYPE]
    ATTN_DT = DT[ATTN_DTYPE]
    V_DT = DT[V_DTYPE]
    NPDT = {"f32r": np.float32, "f16": np.float16}
    Tanh = mybir.ActivationFunctionType.Tanh

    nc = bacc.Bacc("TRN2", target_bir_lowering=False, debug=False,
                   num_devices=NCORES)
    xT_ap = nc.dram_tensor("xT", [DIM, N], PROJ_DT, kind="ExternalInput").ap()
    xv_ap = nc.dram_tensor("xv", [N, HALF], V_DT, kind="ExternalInput").ap()
    wqT_ap = nc.dram_tensor("wqT", [DIM, HALF], PROJ_DT,
                            kind="ExternalInput").ap()
    wkT_ap = nc.dram_tensor("wkT", [DIM, HALF], PROJ_DT,
                            kind="ExternalInput").ap()
    outT_ap = nc.dram_tensor("outT", [HALF, N], F32, kind="ExternalOutput").ap()

    NT = N // 512          # 4 t-chunks of 512
    NJ = N // 128          # 16 j-tiles

    with tile.TileContext(nc) as tc:
        with ExitStack() as ctx:
            const = ctx.enter_context(tc.tile_pool(name="const", bufs=1))
            qk_pool = ctx.enter_context(tc.tile_pool(name="qk", bufs=1))
            tanh_pool = ctx.enter_context(tc.tile_pool(name="tanh", bufs=6))
            stg_pool = ctx.enter_context(tc.tile_pool(name="stg", bufs=4))

            # ---- load inputs (xT first: projections chase its chunks) ----
            xT_sb = const.tile([128, 4 * N], PROJ_DT)
            for ct in range(4):
                nc.sync.dma_start(xT_sb[:, ct * N:(ct + 1) * N],
                                  xT_ap[ct * 128:(ct + 1) * 128, :])
            # wqT/wkT [512, 256] -> [128, 4*256] (c-tile ct at cols ct*256)
            wq_sb = const.tile([128, 4 * HALF], PROJ_DT)
            wk_sb = const.tile([128, 4 * HALF], PROJ_DT)
            for w_sb, w_ap in ((wq_sb, wqT_ap), (wk_sb, wkT_ap)):
                for ct in range(4):
                    nc.sync.dma_start(w_sb[:, ct * HALF:(ct + 1) * HALF],
                                      w_ap[ct * 128:(ct + 1) * 128, :])
            # xv [2048, 256] -> 16 contiguous j-tiles [128, 256]
            xv_sb = []
            for j in range(16):
                t = const.tile([128, HALF], V_DT, tag=f"xv{j}", name=f"xv{j}")
                nc.sync.dma_start(t[:], xv_ap[j * 128:(j + 1) * 128, :])
                xv_sb.append(t)

            # ---- projections + attention ----
            # PSUM: ps_S 3 bufs x [128,1024] (6 banks) + ps_acc 2 x [64,512]
            # (2 banks). Projection groups borrow ps_S slots; head-pair 1's
            # projections are interleaved into pair 0's attention steps so
            # they hide under the tanh stream.
            QT = [qk_pool.tile([128, N], ATTN_DT, tag=f"qt{p}", name=f"qt{p}")
                  for p in range(2)]
            KT = [qk_pool.tile([128, N], ATTN_DT, tag=f"kt{p}", name=f"kt{p}")
                  for p in range(2)]
            ps_S = ctx.enter_context(
                tc.tile_pool(name="ps_S", bufs=3, space="PSUM"))
            ps_acc = ctx.enter_context(
                tc.tile_pool(name="ps_acc", bufs=2, space="PSUM"))

            def proj_group(dst, w_sb, p, t4):
                ps = ps_S.tile([128, 512], F32, tag="S", name="proj_ps")
                for ct in range(4):
                    lhsT = w_sb[:, ct * HALF + p * 128:
                                ct * HALF + (p + 1) * 128]
                    rhs = xT_sb[:, ct * N + t4 * 512: ct * N + t4 * 512 + 512]
                    nc.tensor.matmul(ps[:], lhsT, rhs,
                                     start=(ct == 0), stop=(ct == 3))
                nc.vector.tensor_copy(dst[p][:, t4 * 512:(t4 + 1) * 512],
                                      ps[:])

            # pair-0 projections upfront (K first: the j loop spans all of KT)
            for t4 in range(NT):
                proj_group(KT, wk_sb, 0, t4)
            for t4 in range(NT):
                proj_group(QT, wq_sb, 0, t4)
            p1_groups = [(KT, wk_sb, 1, t4) for t4 in range(NT)] + \
                        [(QT, wq_sb, 1, t4) for t4 in range(NT)]

            stg = {}
            for p in range(2):
                for par in range(2):
                    stg[(p, par)] = stg_pool.tile([64, N], F32, tag="stg",
                                                  name="stg")
            for p in range(2):
                s_idx = 0
                for iq in range(4):          # i-quarter: i cols iq*512..+512
                    acc = [ps_acc.tile([64, 512], F32, tag="acc", name="acc")
                           for par in range(2)]
                    i0 = iq * 512
                    for j in range(NJ):
                        S = ps_S.tile([128, 1024], F32, tag="S", name="S")
                        # row-packed pair: head parity 0 on PE rows 0-63,
                        # parity 1 on rows 64-127
                        nc.tensor.matmul(
                            S[:, 0:512],
                            KT[p][0:64, j * 128:(j + 1) * 128],
                            QT[p][0:64, i0:i0 + 512],
                            start=True, stop=True, tile_position=(0, 0))
                        nc.tensor.matmul(
                            S[:, 512:1024],
                            KT[p][64:128, j * 128:(j + 1) * 128],
                            QT[p][64:128, i0:i0 + 512],
                            start=True, stop=True, tile_position=(64, 0))
                        T = tanh_pool.tile([128, 1024], V_DT, tag="T",
                                           name="T")
                        nc.scalar.activation(T[:], S[:], Tanh)
                        if p == 0 and s_idx % 8 == 2 and s_idx // 8 < 8:
                            proj_group(*p1_groups[s_idx // 8])
                        for par in range(2):
                            lh = 2 * p + par
                            v = xv_sb[j][:, lh * 64:(lh + 1) * 64]
                            nc.tensor.matmul(
                                acc[par][:],
                                v,
                                T[:, par * 512:(par + 1) * 512],
                                start=(j == 0), stop=(j == NJ - 1))
                        s_idx += 1
                    for par in range(2):
                        nc.vector.tensor_copy(
                            stg[(p, par)][:, iq * 512:(iq + 1) * 512],
                            acc[par][:])
                for par in range(2):
                    lh = 2 * p + par
                    nc.sync.dma_start(outT_ap[lh * 64:(lh + 1) * 64, :],
                                      stg[(p, par)][:])

    nc.compile()
    return nc


def _get_built():
    global _built, _built_cfg
    cfg = (PROJ_DTYPE, ATTN_DTYPE, V_DTYPE)
    if _built is None or _built_cfg != cfg:
        _built = _build()
        _built_cfg = cfg
    return _built


def kernel(x, Wq, Wk):
    from concourse.bass_utils import run_bass_kernel_spmd

    x = np.asarray(x, dtype=np.float32)
    Wq = np.asarray(Wq, dtype=np.float32)
    Wk = np.asarray(Wk, dtype=np.float32)

    import ml_dtypes
    proj_np = np.float16 if PROJ_DTYPE == "f16" else np.float32
    v_np = {"f16": np.float16, "bf16": ml_dtypes.bfloat16}[V_DTYPE]

    nc = _get_built()
    in_maps = []
    for c in range(NCORES):
        b, half = c // 2, c % 2
        sl = slice(half * HALF, (half + 1) * HALF)
        in_maps.append({
            "xT": np.ascontiguousarray(x[b].T).astype(proj_np),
            "xv": np.ascontiguousarray(x[b][:, sl]).astype(v_np),
            "wqT": np.ascontiguousarray((SCALE * Wq[sl, :]).T).astype(proj_np),
            "wkT": np.ascontiguousarray(Wk[sl, :].T).astype(proj_np),
        })
    res = run_bass_kernel_spmd(nc, in_maps, core_ids=list(range(NCORES)),
                               trace=TRACE, **TRACE_KW)
    out = np.empty((B, N, DIM), np.float32)
    for c in range(NCORES):
        b, half = c // 2, c % 2
        out[b, :, half * HALF:(half + 1) * HALF] = res.results[c]["outT"].T
    if TRACE:
        kernel.last_results = res
    return out
